# revision 36
# baseline (speedup 1.0000x reference)
"""Trainium2 Bass kernel for nn_AttentionModel (sparse_attention).

8-core distribution:
 - layer-1 convs: tensor-parallel over output channels (128/core), full x input.
   Outputs stay LOCAL (no gather).
 - layer-2 convs: each core computes PARTIAL sums for ALL output channels from
   its local 128-channel stage-1 slice; a ReduceScatter sums the partials and
   hands each core its output-channel shard (so q2 runs at M=128 instead of 32).
 - layer-3 convs: channel-sharded over an AllGather of the layer-2 shards.
 - attention tail: scores+softmax replicated; o and the 1x1 projection are
   POSITION-sharded (each core owns 256 of 2048 query positions), so no big
   gather of o is needed. The per-core beta column-slice is fetched with an
   indirect DMA driven by a per-core index input (keeps the program SPMD).

dtypes: convs/o/proj in float32r (1 cyc/row at N>=512), fp32 PSUM accumulation,
ReduceScatter in fp32; scores matmul + softmax in fp32.
"""
import os
import sys
import numpy as np

for _p in ('/opt/trn_rl_repo', '/root/problem/work'):
    if _p not in sys.path:
        sys.path.insert(0, _p)

import concourse.bass as bass
import concourse.bacc as bacc
import concourse.tile as tile
import concourse.mybir as mybir
from concourse import bass_utils
from concourse.bass_interp import get_hw_module

F32 = mybir.dt.float32
F32R = mybir.dt.float32r
I32 = mybir.dt.int32
AF = mybir.ActivationFunctionType
ALU = mybir.AluOpType
AX = mybir.AxisListType

NCORES = 8
NPOS = 2048
_CACHE = {}


def _lrelu(nc, sb, src_ap, bias_ap, bias3_ap, out_ap, name):
    """out = max(src + b, 0.3*src + 0.3b)  (LeakyReLU 0.3; HW Lrelu ignores alpha).
    Processed in <=1024-wide chunks of a flattened free dim to bound temp size."""
    P = src_ap.shape[0]
    free = int(np.prod(src_ap.shape[1:]))
    if len(src_ap.shape) == 2 and free > 1024:
        for lo in range(0, free, 1024):
            hi = min(lo + 1024, free)
            _lrelu(nc, sb, src_ap[:, lo:hi], bias_ap, bias3_ap, out_ap[:, lo:hi],
                   f"{name}_{lo}")
        return
    s = sb.tile([P, free], F32, name=f"{name}_s", tag="epi_s")
    t = sb.tile([P, free], F32, name=f"{name}_t", tag="epi_t")
    nc.scalar.activation(s[:], src_ap, AF.Identity, bias=bias_ap, scale=1.0)
    nc.scalar.activation(t[:], src_ap, AF.Identity, bias=bias3_ap, scale=0.3)
    nc.vector.tensor_tensor(out_ap, s[:], t[:], op=ALU.max)


def build_program():
    nc = bacc.Bacc("TRN2", target_bir_lowering=False, debug=False,
                   enable_asserts=True, num_devices=NCORES)

    xpad_d = nc.dram_tensor("xpad", [16, 128, 34 * 66], F32, kind="ExternalInput")
    xdec_d = nc.dram_tensor("xdec", [16, 128, 4 * 17 * 33], F32, kind="ExternalInput")
    w1q_d = nc.dram_tensor("w1q", [16, 128, 1152], F32, kind="ExternalInput")
    w1k_d = nc.dram_tensor("w1k", [16, 128, 1152], F32, kind="ExternalInput")
    w1v_d = nc.dram_tensor("w1v", [16, 128, 1152], F32, kind="ExternalInput")
    w2q_d = nc.dram_tensor("w2q", [128, 2304], F32, kind="ExternalInput")
    w2k_d = nc.dram_tensor("w2k", [128, 2304], F32, kind="ExternalInput")
    w2v_d = nc.dram_tensor("w2v", [2, 128, 4608], F32, kind="ExternalInput")
    w3q_d = nc.dram_tensor("w3q", [2, 128, 288], F32, kind="ExternalInput")
    w3k_d = nc.dram_tensor("w3k", [2, 128, 288], F32, kind="ExternalInput")
    w3v_d = nc.dram_tensor("w3v", [8, 128, 1152], F32, kind="ExternalInput")
    wp_d = nc.dram_tensor("wp", [8, 128, 1024], F32, kind="ExternalInput")
    bias_d = nc.dram_tensor("bias", [128, 28], F32, kind="ExternalInput")
    bidx_d = nc.dram_tensor("bidx", [65, 1], I32, kind="ExternalInput")
    out_d = nc.dram_tensor("out_shard", [1024, 256], F32, kind="ExternalOutput")
    ident_d = nc.inline_tensor(np.eye(128, dtype=np.float32), name="ident")

    RG = [list(range(NCORES))]

    with tile.TileContext(nc) as tc:
        with (
            tc.tile_pool(name="dram", bufs=1, space="DRAM") as dram,
            tc.tile_pool(name="wpool", bufs=2) as wpool,
            tc.tile_pool(name="xpool", bufs=2) as xpool,
            tc.tile_pool(name="opool", bufs=1) as opool,
            tc.tile_pool(name="ppool", bufs=1, space="PSUM") as ppool,
            tc.tile_pool(name="misc", bufs=1) as misc,
        ):
            # collective buffers
            rsa_in = dram.tile([8, 17920], F32)                  # k2/v2 partials
            rsa_out = dram.tile([17920], F32)
            rsb_in = dram.tile([8, 65536], F32)                  # q2 partials
            rsb_out = dram.tile([65536], F32)
            ag2a_in = dram.tile([17920], F32R)                   # k2/v2 shards
            ag2a_out = dram.tile([8, 17920], F32R, addr_space="Shared")
            ag2b_in = dram.tile([32, 2244], F32R)                # q2 shard (padded)
            ag2b_out = dram.tile([256, 2244], F32R, addr_space="Shared")
            ag3_in = dram.tile([32, 2373], F32)                  # q3 | k3 | v3
            ag3_out = dram.tile([256, 2373], F32, addr_space="Shared")
            beta_dram = dram.tile([65, 2048], F32)

            biases = misc.tile([128, 28], F32)
            nc.sync.dma_start(biases[:], bias_d.ap())
            bcol = lambda j: biases[:, j:j + 1]

            # tiny warmup collective: pays the first-collective setup cost
            # while stage 1 computes
            warm_in = dram.tile([128, 4], F32)
            warm_out = dram.tile([1024, 4], F32, addr_space="Shared")
            nc.sync.dma_start(warm_in[:], bias_d.ap()[:, 0:4])
            nc.gpsimd.collective_compute("AllGather", ALU.bypass, replica_groups=RG,
                                         ins=[warm_in.opt()], outs=[warm_out.opt()])

            # ============ STAGE 1: layer-1 convs (single pass, PE-bound) =====
            q1_ps = ppool.tile([128, 2048], F32, name="q1_ps", tag="pbig")
            k1_ps = ppool.tile([128, 512], F32, name="k1_ps", tag="pk")
            v1_ps = ppool.tile([128, 512], F32, name="v1_ps", tag="pv")
            for ic in range(16):
                xp = xpool.tile([128, 34 * 66], F32R, name="xp", tag="xbig", bufs=3)
                nc.gpsimd.dma_start(xp[:], xpad_d.ap()[ic])     # cast f32 -> f32r
                xd = xpool.tile([128, 4 * 17 * 33], F32R, name="xd", tag="xdec", bufs=3)
                nc.gpsimd.dma_start(xd[:], xdec_d.ap()[ic])
                wq = wpool.tile([128, 1152], F32R, name="wq", tag="wA")
                nc.gpsimd.dma_start(wq[:], w1q_d.ap()[ic])
                wk = wpool.tile([128, 1152], F32R, name="wk", tag="wB")
                nc.gpsimd.dma_start(wk[:], w1k_d.ap()[ic])
                wv = wpool.tile([128, 1152], F32R, name="wv", tag="wC")
                nc.gpsimd.dma_start(wv[:], w1v_d.ap()[ic])
                x3 = xp.rearrange("c (h w) -> c h w", h=34)
                xd4 = xd.rearrange("c (f h w) -> c f h w", f=4, h=17)
                first, last = (ic == 0), (ic == 15)
                for tap in range(9):
                    dy, dx = tap // 3, tap % 3
                    ph = 2 * (dy % 2) + (dx % 2)
                    win2 = xd4[:, ph:ph + 1, dy // 2: dy // 2 + 16, dx // 2: dx // 2 + 32]
                    nc.tensor.matmul(k1_ps[:], wk[:, tap * 128:tap * 128 + 128], win2,
                                     start=(first and tap == 0), stop=(last and tap == 8))
                    nc.tensor.matmul(v1_ps[:], wv[:, tap * 128:tap * 128 + 128], win2,
                                     start=(first and tap == 0), stop=(last and tap == 8))
                for tap in range(9):
                    dy, dx = tap // 3, tap % 3
                    wq_t = wq[:, tap * 128:tap * 128 + 128]
                    for t in range(4):
                        win = x3[:, 8 * t + dy: 8 * t + dy + 8, dx: dx + 64]
                        nc.tensor.matmul(q1_ps[:, 512 * t: 512 * t + 512], wq_t, win,
                                         start=(first and tap == 0),
                                         stop=(last and tap == 8))

            k1_sb = opool.tile([128, 15 * 33], F32R, name="k1_sb", tag="okv")
            k1o = k1_sb.rearrange("c (h w) -> c h w", h=15)
            k1g = k1_ps.rearrange("c (h w) -> c h w", h=16)
            _lrelu(nc, misc, k1g[:, 0:15, 0:31], bcol(1), bcol(11), k1o[:, :, 0:31], "k1e")
            nc.vector.tensor_copy(k1o[:, :, 31:33], k1o[:, :, 0:2])
            v1_sb = opool.tile([128, 15 * 33], F32R, name="v1_sb", tag="ovv")
            v1o = v1_sb.rearrange("c (h w) -> c h w", h=15)
            v1g = v1_ps.rearrange("c (h w) -> c h w", h=16)
            _lrelu(nc, misc, v1g[:, 0:15, 0:31], bcol(2), bcol(12), v1o[:, :, 0:31], "v1e")
            nc.vector.tensor_copy(v1o[:, :, 31:33], v1o[:, :, 0:2])

            q1_sb = opool.tile([128, 34 * 66], F32R, name="q1_sb", tag="obig")
            q1o = q1_sb.rearrange("c (h w) -> c h w", h=34)
            q1v = q1_ps.rearrange("c (h w) -> c h w", h=32)
            _lrelu(nc, misc, q1v[:, 0:16, :], bcol(0), bcol(10), q1o[:, 1:17, 1:65], "q1e0")
            _lrelu(nc, misc, q1v[:, 16:32, :], bcol(0), bcol(10), q1o[:, 17:33, 1:65], "q1e1")
            nc.vector.tensor_copy(q1o[:, 0:1, 1:65], q1o[:, 2:3, 1:65])
            nc.vector.tensor_copy(q1o[:, 33:34, 1:65], q1o[:, 31:32, 1:65])
            nc.vector.tensor_copy(q1o[:, :, 0:1], q1o[:, :, 64:65])
            nc.vector.tensor_copy(q1o[:, :, 65:66], q1o[:, :, 1:2])


            # k2/v2 partial convs + RSa, issued now so they overlap the q2 partials
            w2k = opool.tile([128, 2304], F32R, name="w2k", tag="wk2")
            nc.gpsimd.dma_start(w2k[:], w2k_d.ap())
            for cc in range(2):
                kp = ppool.tile([128, 112], F32, name="kp", tag="pk")
                for tap in range(9):
                    dy, dx = tap // 3, tap % 3
                    wink = k1o[:, dy: dy + 13: 2, dx: dx + 31: 2]
                    nc.tensor.matmul(kp[:], w2k[:, tap * 256 + 128 * cc: tap * 256 + 128 * cc + 128],
                                     wink, start=(tap == 0), stop=(tap == 8))
                kps = misc.tile([128, 112], F32, name="kps", tag="rss")
                nc.scalar.copy(kps[:], kp[:])
                dst = rsa_in[4 * cc:4 * cc + 4, 0:3584].rearrange("r (c p) -> r c p", c=32)
                nc.sync.dma_start(dst, kps[:])
            for vh in range(2):
                w2v = opool.tile([128, 4608], F32R, name="w2v", tag="wbig")
                nc.gpsimd.dma_start(w2v[:], w2v_d.ap()[vh])
                for cc4 in range(4):
                    cc = 4 * vh + cc4
                    vp = ppool.tile([128, 112], F32, name="vp", tag="pv")
                    for tap in range(9):
                        dy, dx = tap // 3, tap % 3
                        winv = v1o[:, dy: dy + 13: 2, dx: dx + 31: 2]
                        nc.tensor.matmul(vp[:], w2v[:, tap * 512 + 128 * cc4: tap * 512 + 128 * cc4 + 128],
                                         winv, start=(tap == 0), stop=(tap == 8))
                    vps = misc.tile([128, 112], F32, name="vps", tag="rss")
                    nc.scalar.copy(vps[:], vp[:])
                    nc.sync.dma_start(rsa_in[cc, 3584:17920].rearrange("(c p) -> c p", c=128), vps[:])
            nc.gpsimd.collective_compute("ReduceScatter", ALU.add, replica_groups=RG,
                                         ins=[rsa_in.opt()], outs=[rsa_out.opt()])

            # k2/v2 shard epilogues + AG2a (all overlap the q1 pass below)
            k2r = misc.tile([32, 112], F32, name="k2r", tag="rsl")
            nc.sync.dma_start(k2r[:], rsa_out[0:3584].rearrange("(c p) -> c p", c=32))
            v2r = misc.tile([128, 112], F32, name="v2r", tag="rsl2")
            nc.sync.dma_start(v2r[:], rsa_out[3584:17920].rearrange("(c p) -> c p", c=128))
            k2_sb = opool.tile([32, 112], F32R, name="k2_sb", tag="okv2")
            k2o = k2_sb.rearrange("c (h w) -> c h w", h=7)
            k2rg = k2r.rearrange("c (h w) -> c h w", h=7)
            _lrelu(nc, misc, k2rg[:, :, 0:15], bcol(4)[0:32], bcol(14)[0:32], k2o[:, :, 0:15], "k2e")
            nc.vector.tensor_copy(k2o[:, :, 15:16], k2o[:, :, 0:1])
            v2_sb = opool.tile([128, 112], F32R, name="v2_sb", tag="ovv2")
            v2o = v2_sb.rearrange("c (h w) -> c h w", h=7)
            v2rg = v2r.rearrange("c (h w) -> c h w", h=7)
            _lrelu(nc, misc, v2rg[:, :, 0:15], bcol(5), bcol(15), v2o[:, :, 0:15], "v2e")
            nc.vector.tensor_copy(v2o[:, :, 15:16], v2o[:, :, 0:1])
            nc.sync.dma_start(ag2a_in[0:3584].rearrange("(c p) -> c p", c=32), k2_sb[:])
            nc.sync.dma_start(ag2a_in[3584:17920].rearrange("(c p) -> c p", c=128), v2_sb[:])
            nc.gpsimd.collective_compute("AllGather", ALU.bypass, replica_groups=RG,
                                         ins=[ag2a_in.opt()], outs=[ag2a_out.opt()])

            # q2 partials (M=128!) from local q1
            w2q = opool.tile([128, 2304], F32R, name="w2q", tag="wq2")
            nc.gpsimd.dma_start(w2q[:], w2q_d.ap())
            for cc in range(2):
                qp = ppool.tile([128, 2048], F32, name="qp", tag="pbig")
                for tap in range(9):
                    dy, dx = tap // 3, tap % 3
                    wslc = w2q[:, tap * 256 + 128 * cc: tap * 256 + 128 * cc + 128]
                    for t in range(4):
                        win = q1o[:, 8 * t + dy: 8 * t + dy + 8, dx: dx + 64]
                        nc.tensor.matmul(qp[:, 512 * t:512 * t + 512], wslc, win,
                                         start=(tap == 0), stop=(tap == 8))
                qps = misc.tile([128, 2048], F32, name="qps", tag="rssb")
                nc.scalar.copy(qps[:], qp[:])
                dst = rsb_in[4 * cc:4 * cc + 4, :].rearrange("r (c p) -> r c p", c=32)
                nc.sync.dma_start(dst, qps[:])
            nc.gpsimd.collective_compute("ReduceScatter", ALU.add, replica_groups=RG,
                                         ins=[rsb_in.opt()], outs=[rsb_out.opt()])

            q2r = misc.tile([32, 2048], F32, name="q2r", tag="rssb")
            nc.sync.dma_start(q2r[:], rsb_out[:].rearrange("(c p) -> c p", c=32))
            q2_sb = opool.tile([32, 34 * 66], F32R, name="q2_sb", tag="obig")
            q2o = q2_sb.rearrange("c (h w) -> c h w", h=34)
            q2rv = q2r.rearrange("c (h w) -> c h w", h=32)
            _lrelu(nc, misc, q2rv[:, 0:16, :], bcol(3)[0:32], bcol(13)[0:32], q2o[:, 1:17, 1:65], "q2e0")
            _lrelu(nc, misc, q2rv[:, 16:32, :], bcol(3)[0:32], bcol(13)[0:32], q2o[:, 17:33, 1:65], "q2e1")
            nc.vector.tensor_copy(q2o[:, 0:1, 1:65], q2o[:, 2:3, 1:65])
            nc.vector.tensor_copy(q2o[:, 33:34, 1:65], q2o[:, 31:32, 1:65])
            nc.vector.tensor_copy(q2o[:, :, 0:1], q2o[:, :, 64:65])
            nc.vector.tensor_copy(q2o[:, :, 65:66], q2o[:, :, 1:2])
            nc.sync.dma_start(ag2b_in[:], q2_sb[:])
            nc.gpsimd.collective_compute("AllGather", ALU.bypass, replica_groups=RG,
                                         ins=[ag2b_in.opt()], outs=[ag2b_out.opt()])

            # projection weights: load+convert early so they overlap stage 3
            wpts = []
            for half in range(2):
                wpt = opool.tile([128, 4096], F32R, name="wpt", tag=f"wpt{half}")
                for qtr in range(2):
                    wstg = misc.tile([128, 2048], F32, name="wstg", tag="wstg")
                    nc.sync.dma_start(wstg.rearrange("b (a c) -> b a c", a=2),
                                      wp_d.ap()[4 * half + 2 * qtr: 4 * half + 2 * qtr + 2].rearrange("a b c -> b a c"))
                    nc.vector.tensor_copy(wpt[:, 2048 * qtr:2048 * qtr + 2048], wstg[:])
                wpts.append(wpt)

            # ============ STAGE 3: k3/v3 first (need ag2a), then q3 ==========
            k3_ps = ppool.tile([32, 70], F32, name="k3_ps", tag="pk")
            v3_ps = ppool.tile([128, 70], F32, name="v3_ps", tag="pv")
            k2rr = ag2a_out[:, 0:3584].rearrange("r (c p) -> r c p", c=32)
            v2rr = ag2a_out[:, 3584:17920].rearrange("r (c p) -> r c p", c=128)
            for jc in range(2):
                k2c = xpool.tile([128, 112], F32R, name="k2c", tag="k2c")
                nc.sync.dma_start(k2c[:], k2rr[4 * jc:4 * jc + 4])
                k2c3 = k2c.rearrange("c (h w) -> c h w", h=7)
                w3k = wpool.tile([128, 288], F32R, name="w3k", tag="wB")
                nc.gpsimd.dma_start(w3k[:], w3k_d.ap()[jc])
                for tap in range(9):
                    dy, dx = tap // 3, tap % 3
                    # k3: out grid (5, 14), cols 0..12 valid; N=70
                    wink = k2c3[:, dy: dy + 5, dx: dx + 14]
                    nc.tensor.matmul(k3_ps[:], w3k[:, tap * 32:tap * 32 + 32], wink,
                                     start=(jc == 0 and tap == 0), stop=(jc == 1 and tap == 8))
            for ic in range(8):
                v2c = xpool.tile([128, 112], F32R, name="v2c", tag="v2c")
                nc.sync.dma_start(v2c[:], v2rr[ic])
                v2c3 = v2c.rearrange("c (h w) -> c h w", h=7)
                w3v = wpool.tile([128, 1152], F32R, name="w3v", tag="wC")
                nc.gpsimd.dma_start(w3v[:], w3v_d.ap()[ic])
                for tap in range(9):
                    dy, dx = tap // 3, tap % 3
                    winv = v2c3[:, dy: dy + 5, dx: dx + 14]
                    nc.tensor.matmul(v3_ps[:], w3v[:, tap * 128:tap * 128 + 128], winv,
                                     start=(ic == 0 and tap == 0), stop=(ic == 7 and tap == 8))

            k3g = k3_ps.rearrange("c (h w) -> c h w", h=5)
            k3_sb = opool.tile([32, 65], F32, name="k3_sb", tag="okv")
            _lrelu(nc, misc, k3g[:, :, 0:13], bcol(7)[0:32], bcol(17)[0:32], k3_sb[:], "k3e")
            v3g = v3_ps.rearrange("c (h w) -> c h w", h=5)
            v3_sb = opool.tile([128, 65], F32, name="v3_sb", tag="v3sb")
            _lrelu(nc, misc, v3g[:, :, 0:13], bcol(8), bcol(18), v3_sb[:], "v3e")

            q3_ps = ppool.tile([32, 2048], F32, name="q3_ps", tag="pbig")
            for jc in range(2):
                q2p_t = xpool.tile([128, 34 * 66], F32R, name="q2p", tag="xbig", bufs=3)
                nc.sync.dma_start(q2p_t[:], ag2b_out[128 * jc:128 * jc + 128])
                q2p = q2p_t.rearrange("c (h w) -> c h w", h=34)
                w3q = wpool.tile([128, 288], F32R, name="w3q", tag="wA")
                nc.gpsimd.dma_start(w3q[:], w3q_d.ap()[jc])
                first, last = (jc == 0), (jc == 1)
                for tap in range(9):
                    dy, dx = tap // 3, tap % 3
                    for t in range(4):
                        win = q2p[:, 8 * t + dy: 8 * t + dy + 8, dx: dx + 64]
                        nc.tensor.matmul(q3_ps[:, 512 * t:512 * t + 512],
                                         w3q[:, tap * 32:tap * 32 + 32], win,
                                         start=(first and tap == 0), stop=(last and tap == 8))

            q3_sb = opool.tile([32, 2048], F32, name="q3_sb", tag="obig")
            _lrelu(nc, misc, q3_ps[:], bcol(6)[0:32], bcol(16)[0:32], q3_sb[:], "q3e")
            nc.sync.dma_start(ag3_in[:, 0:2048], q3_sb[:])
            nc.sync.dma_start(ag3_in[:, 2048:2113], k3_sb[:])
            # v3 packed as channel p -> (row p//4, col-block p%4)
            nc.sync.dma_start(ag3_in[:, 2113:2373].rearrange("c (a p) -> c a p", a=4), v3_sb[:])
            nc.gpsimd.collective_compute("AllGather", ALU.bypass, replica_groups=RG,
                                         ins=[ag3_in.opt()], outs=[ag3_out.opt()])

            # ============ STAGE 4: attention + position-sharded projection ===
            sc_ps = ppool.tile([65, 2048], F32, name="sc_ps", tag="pbig")
            for jc in range(2):
                q3f = opool.tile([128, 2048], F32, name="q3f", tag="wq2")
                nc.sync.dma_start(q3f[:], ag3_out[128 * jc:128 * jc + 128, 0:2048])
                k3f = misc.tile([128, 65], F32, name="k3f", tag="k3f", bufs=2)
                nc.sync.dma_start(k3f[:], ag3_out[128 * jc:128 * jc + 128, 2048:2113])
                for t in range(4):
                    nc.tensor.matmul(sc_ps[:, 512 * t:512 * t + 512], k3f[:],
                                     q3f[:, 512 * t:512 * t + 512],
                                     start=(jc == 0), stop=(jc == 1))

            negmax = misc.tile([65, 1], F32)
            nc.vector.reduce_max(negmax[:], sc_ps[:], axis=AX.X, negate=True)
            esum = misc.tile([65, 1], F32)
            bexp = misc.tile([65, 2048], F32)
            nc.scalar.activation(bexp[:], sc_ps[:], AF.Exp, bias=negmax[:, 0:1],
                                 accum_out=esum[:, 0:1])
            rsum = misc.tile([65, 1], F32)
            nc.vector.reciprocal(rsum[:], esum[:])
            # ship UNnormalized exp; the 1/sum(m) factor is folded into v3^T
            # below (commutes through the m-contraction of the o matmul)
            nc.sync.dma_start(beta_dram[:], bexp[:])


            # indirect gather of MY 256 beta columns: row (m, blk) of (520, 256)
            bidx = misc.tile([65, 1], I32)
            nc.sync.dma_start(bidx[:], bidx_d.ap())
            betaB = misc.tile([65, 256], F32)
            nc.gpsimd.indirect_dma_start(
                out=betaB[:], out_offset=None,
                in_=beta_dram.rearrange("m (b p) -> (m b) p", b=8),
                in_offset=bass.IndirectOffsetOnAxis(ap=bidx[:, 0:1], axis=0))

            # v3^T chunks (65, 128) for all 1024 v-channels
            ident = misc.tile([128, 128], F32)
            nc.sync.dma_start(ident[:], ident_d.ap())
            v3r = ag3_out[:, 2113:2373].rearrange("r (a p) -> r a p", a=4)
            v3ta = misc.tile([65, 1024], F32, name="v3ta")
            for i in range(8):
                v3f = misc.tile([128, 65], F32, name="v3f", tag="v3f", bufs=2)
                nc.sync.dma_start(v3f[:], v3r[32 * i:32 * i + 32])
                tps = ppool.tile([65, 128], F32, name="tps", tag="pk")
                nc.tensor.transpose(tps[:], v3f[:, 0:65], ident[:])
                nc.scalar.copy(v3ta[:, 128 * i:128 * i + 128], tps[:])
            nc.vector.tensor_scalar_mul(v3ta[:], v3ta[:], rsum[:, 0:1])

            # o chunks (128 v-ch, 256 pos) then projection (all 1024 out-ch)
            oia = misc.tile([128, 2048], F32R, name="oia", tag="rssb")
            for i in range(8):
                ops = ppool.tile([128, 256], F32, name="ops", tag="pv")
                nc.tensor.matmul(ops[:], v3ta[:, 128 * i:128 * i + 128], betaB[:],
                                 start=True, stop=True)
                nc.scalar.copy(oia[:, 256 * i:256 * i + 256], ops[:])
            acca = misc.tile([128, 2048], F32, name="acca", tag="bexp")
            for half in range(2):
                wpt = wpts[half]
                for cc in range(8):
                    out_ps = ppool.tile([128, 256], F32, name="out_ps", tag="pk")
                    for c4 in range(4):
                        cik = 4 * half + c4
                        nc.tensor.matmul(out_ps[:],
                                         wpt[:, 1024 * c4 + 128 * cc: 1024 * c4 + 128 * cc + 128],
                                         oia[:, 256 * cik:256 * cik + 256],
                                         start=(c4 == 0), stop=(c4 == 3))
                    if half == 0:
                        nc.vector.tensor_scalar_add(acca[:, 256 * cc:256 * cc + 256],
                                                    out_ps[:], bcol(20 + cc))
                    else:
                        out_sb = misc.tile([128, 256], F32, name="out_sb", tag="osb", bufs=2)
                        nc.vector.tensor_tensor(out_sb[:], acca[:, 256 * cc:256 * cc + 256],
                                                out_ps[:], op=ALU.add)
                        nc.sync.dma_start(out_d.ap()[128 * cc:128 * cc + 128], out_sb[:])

    nc.compile()
    nc.m = get_hw_module(nc.m)
    return nc


def _prep_inputs(x, qw1, qb1, qw2, qb2, qw3, qb3, kw1, kb1, kw2, kb2, kw3, kb3,
                 vw1, vb1, vw2, vb2, vw3, vb3, pw, pb):
    f = np.float32
    x = np.ascontiguousarray(np.asarray(x).reshape(2048, 32, 64), dtype=f)
    xp = np.concatenate([x[:, 1:2], x, x[:, 30:31]], axis=1)
    xp = np.concatenate([xp[:, :, -1:], xp, xp[:, :, :1]], axis=2)
    xpad = np.ascontiguousarray(xp.reshape(16, 128, 34 * 66))
    xdec = np.zeros((16, 128, 4, 17, 33), f)
    xr = x.reshape(16, 128, 32, 64)
    for py in range(2):
        for px in range(2):
            xdec[:, :, 2 * py + px, 0:16, 0:32] = xr[:, :, py::2, px::2]
    xdec = np.ascontiguousarray(xdec.reshape(16, 128, 4 * 17 * 33))

    def conv_w(wt, co_lo, co_n, nchunk):
        ws = np.asarray(wt)[co_lo:co_lo + co_n]           # (co_n, Ci, 3, 3)
        ci = ws.shape[1]
        a = ws.reshape(co_n, nchunk, ci // nchunk, 9)     # (co, ck, ci, tap)
        a = a.transpose(1, 2, 3, 0)                       # (ck, ci, tap, co)
        return np.ascontiguousarray(a.reshape(nchunk, ci // nchunk, 9 * co_n), dtype=f)

    def conv_w_ci(wt, ci_lo):
        # full out-channels, my 128 input channels -> (128ci, 9*co)
        ws = np.asarray(wt)[:, ci_lo:ci_lo + 128]         # (co, 128, 3, 3)
        co = ws.shape[0]
        a = ws.reshape(co, 128, 9).transpose(1, 2, 0)     # (ci, tap, co)
        return np.ascontiguousarray(a.reshape(128, 9 * co), dtype=f)

    in_maps = []
    for c in range(NCORES):
        m = {"xpad": xpad, "xdec": xdec}
        m["w1q"] = conv_w(qw1, 128 * c, 128, 16)
        m["w1k"] = conv_w(kw1, 128 * c, 128, 16)
        m["w1v"] = conv_w(vw1, 128 * c, 128, 16)
        m["w2q"] = conv_w_ci(qw2, 128 * c)
        m["w2k"] = conv_w_ci(kw2, 128 * c)
        wv2 = np.asarray(vw2)[:, 128 * c:128 * c + 128]        # (1024co, 128ci, 3, 3)
        wv2 = wv2.reshape(2, 512, 128, 9).transpose(0, 2, 3, 1)  # (half, ci, tap, co512)
        m["w2v"] = np.ascontiguousarray(wv2.reshape(2, 128, 4608), dtype=f)
        m["w3q"] = conv_w(qw3, 32 * c, 32, 2)
        m["w3k"] = conv_w(kw3, 32 * c, 32, 2)
        m["w3v"] = conv_w(vw3, 128 * c, 128, 8)
        m["wp"] = np.ascontiguousarray(
            np.asarray(pw)[:, :, 0, 0].T.reshape(8, 128, 1024), dtype=f)
        bias = np.zeros((128, 28), f)
        bias[:, 0] = qb1[128 * c:128 * c + 128]
        bias[:, 1] = kb1[128 * c:128 * c + 128]
        bias[:, 2] = vb1[128 * c:128 * c + 128]
        bias[0:32, 3] = qb2[32 * c:32 * c + 32]
        bias[0:32, 4] = kb2[32 * c:32 * c + 32]
        bias[:, 5] = vb2[128 * c:128 * c + 128]
        bias[0:32, 6] = qb3[32 * c:32 * c + 32]
        bias[0:32, 7] = kb3[32 * c:32 * c + 32]
        bias[:, 8] = vb3[128 * c:128 * c + 128]
        bias[:, 10:19] = 0.3 * bias[:, 0:9]
        for j in range(8):
            bias[:, 20 + j] = pb[128 * j:128 * j + 128]
        m["bias"] = bias
        m["bidx"] = np.arange(65, dtype=np.int32).reshape(65, 1) * 8 + c
        in_maps.append(m)
    return in_maps


LAST_RESULT = None


def kernel(**inputs):
    global LAST_RESULT
    if "nc" not in _CACHE:
        _CACHE["nc"] = build_program()
    nc = _CACHE["nc"]
    in_maps = _prep_inputs(**{k: np.asarray(v) for k, v in inputs.items()})
    res = bass_utils.run_bass_kernel_spmd(nc, in_maps, core_ids=list(range(NCORES)))
    LAST_RESULT = res
    out = np.empty((1024, 32, 64), np.float32)
    for c in range(NCORES):
        out[:, 4 * c:4 * c + 4, :] = res.results[c]["out_shard"].reshape(1024, 4, 64)
    return np.ascontiguousarray(out.reshape(1, 1024, 32, 64))


# revision 37
# speedup vs baseline: 1.0026x; 1.0026x over previous
"""Trainium2 Bass kernel for nn_AttentionModel (sparse_attention).

8-core distribution:
 - layer-1 convs: tensor-parallel over output channels (128/core), full x input.
   Outputs stay LOCAL (no gather).
 - layer-2 convs: each core computes PARTIAL sums for ALL output channels from
   its local 128-channel stage-1 slice; a ReduceScatter sums the partials and
   hands each core its output-channel shard (so q2 runs at M=128 instead of 32).
 - layer-3 convs: channel-sharded over an AllGather of the layer-2 shards.
 - attention tail: scores+softmax replicated; o and the 1x1 projection are
   POSITION-sharded (each core owns 256 of 2048 query positions), so no big
   gather of o is needed. The per-core beta column-slice is fetched with an
   indirect DMA driven by a per-core index input (keeps the program SPMD).

dtypes: convs/o/proj in float32r (1 cyc/row at N>=512), fp32 PSUM accumulation,
ReduceScatter in fp32; scores matmul + softmax in fp32.
"""
import os
import sys
import numpy as np

for _p in ('/opt/trn_rl_repo', '/root/problem/work'):
    if _p not in sys.path:
        sys.path.insert(0, _p)

import concourse.bass as bass
import concourse.bacc as bacc
import concourse.tile as tile
import concourse.mybir as mybir
from concourse import bass_utils
from concourse.bass_interp import get_hw_module

F32 = mybir.dt.float32
F32R = mybir.dt.float32r
I32 = mybir.dt.int32
AF = mybir.ActivationFunctionType
ALU = mybir.AluOpType
AX = mybir.AxisListType

NCORES = 8
NPOS = 2048
_CACHE = {}


def _lrelu(nc, sb, src_ap, bias_ap, bias3_ap, out_ap, name):
    """out = max(src + b, 0.3*src + 0.3b)  (LeakyReLU 0.3; HW Lrelu ignores alpha).
    Processed in <=1024-wide chunks of a flattened free dim to bound temp size."""
    P = src_ap.shape[0]
    free = int(np.prod(src_ap.shape[1:]))
    if len(src_ap.shape) == 2 and free > 1024:
        for lo in range(0, free, 1024):
            hi = min(lo + 1024, free)
            _lrelu(nc, sb, src_ap[:, lo:hi], bias_ap, bias3_ap, out_ap[:, lo:hi],
                   f"{name}_{lo}")
        return
    s = sb.tile([P, free], F32, name=f"{name}_s", tag="epi_s")
    t = sb.tile([P, free], F32, name=f"{name}_t", tag="epi_t")
    nc.scalar.activation(s[:], src_ap, AF.Identity, bias=bias_ap, scale=1.0)
    nc.scalar.activation(t[:], src_ap, AF.Identity, bias=bias3_ap, scale=0.3)
    nc.vector.tensor_tensor(out_ap, s[:], t[:], op=ALU.max)


def build_program():
    nc = bacc.Bacc("TRN2", target_bir_lowering=False, debug=False,
                   enable_asserts=True, num_devices=NCORES)

    xpad_d = nc.dram_tensor("xpad", [16, 128, 34 * 66], F32, kind="ExternalInput")
    xdec_d = nc.dram_tensor("xdec", [16, 128, 4 * 17 * 33], F32, kind="ExternalInput")
    w1q_d = nc.dram_tensor("w1q", [16, 128, 1152], F32, kind="ExternalInput")
    w1k_d = nc.dram_tensor("w1k", [16, 128, 1152], F32, kind="ExternalInput")
    w1v_d = nc.dram_tensor("w1v", [16, 128, 1152], F32, kind="ExternalInput")
    w2q_d = nc.dram_tensor("w2q", [128, 2304], F32, kind="ExternalInput")
    w2k_d = nc.dram_tensor("w2k", [128, 2304], F32, kind="ExternalInput")
    w2v_d = nc.dram_tensor("w2v", [2, 128, 4608], F32, kind="ExternalInput")
    w3q_d = nc.dram_tensor("w3q", [2, 128, 288], F32, kind="ExternalInput")
    w3k_d = nc.dram_tensor("w3k", [2, 128, 288], F32, kind="ExternalInput")
    w3v_d = nc.dram_tensor("w3v", [8, 128, 1152], F32, kind="ExternalInput")
    wp_d = nc.dram_tensor("wp", [8, 128, 1024], F32, kind="ExternalInput")
    bias_d = nc.dram_tensor("bias", [128, 28], F32, kind="ExternalInput")
    bidx_d = nc.dram_tensor("bidx", [65, 1], I32, kind="ExternalInput")
    out_d = nc.dram_tensor("out_shard", [1024, 256], F32, kind="ExternalOutput")
    ident_d = nc.inline_tensor(np.eye(128, dtype=np.float32), name="ident")

    RG = [list(range(NCORES))]

    with tile.TileContext(nc) as tc:
        with (
            tc.tile_pool(name="dram", bufs=1, space="DRAM") as dram,
            tc.tile_pool(name="wpool", bufs=2) as wpool,
            tc.tile_pool(name="xpool", bufs=2) as xpool,
            tc.tile_pool(name="opool", bufs=1) as opool,
            tc.tile_pool(name="ppool", bufs=1, space="PSUM") as ppool,
            tc.tile_pool(name="misc", bufs=1) as misc,
        ):
            # collective buffers
            rsa_in = dram.tile([8, 17920], F32)                  # k2/v2 partials
            rsa_out = dram.tile([17920], F32)
            rsb_in = dram.tile([8, 65536], F32)                  # q2 partials
            rsb_out = dram.tile([65536], F32)
            ag2a_in = dram.tile([17920], F32R)                   # k2/v2 shards
            ag2a_out = dram.tile([8, 17920], F32R, addr_space="Shared")
            ag2b_in = dram.tile([32, 2244], F32R)                # q2 shard (padded)
            ag2b_out = dram.tile([256, 2244], F32R, addr_space="Shared")
            ag3_in = dram.tile([32, 2373], F32)                  # q3 | k3 | v3
            ag3_out = dram.tile([256, 2373], F32, addr_space="Shared")
            beta_dram = dram.tile([65, 2048], F32)

            biases = misc.tile([128, 28], F32)
            nc.sync.dma_start(biases[:], bias_d.ap())
            bcol = lambda j: biases[:, j:j + 1]

            # tiny warmup collective: pays the first-collective setup cost
            # while stage 1 computes
            warm_in = dram.tile([128, 4], F32)
            warm_out = dram.tile([1024, 4], F32, addr_space="Shared")
            nc.sync.dma_start(warm_in[:], bias_d.ap()[:, 0:4])
            nc.gpsimd.collective_compute("AllGather", ALU.bypass, replica_groups=RG,
                                         ins=[warm_in.opt()], outs=[warm_out.opt()])

            # ============ STAGE 1: layer-1 convs (single pass, PE-bound) =====
            q1_ps = ppool.tile([128, 2048], F32, name="q1_ps", tag="pbig")
            k1_ps = ppool.tile([128, 512], F32, name="k1_ps", tag="pk")
            v1_ps = ppool.tile([128, 512], F32, name="v1_ps", tag="pv")
            for ic in range(16):
                xp = xpool.tile([128, 34 * 66], F32R, name="xp", tag="xbig")
                nc.gpsimd.dma_start(xp[:], xpad_d.ap()[ic])     # cast f32 -> f32r
                xd = xpool.tile([128, 4 * 17 * 33], F32R, name="xd", tag="xdec")
                nc.gpsimd.dma_start(xd[:], xdec_d.ap()[ic])
                wq = wpool.tile([128, 1152], F32R, name="wq", tag="wA")
                nc.gpsimd.dma_start(wq[:], w1q_d.ap()[ic])
                wk = wpool.tile([128, 1152], F32R, name="wk", tag="wB")
                nc.gpsimd.dma_start(wk[:], w1k_d.ap()[ic])
                wv = wpool.tile([128, 1152], F32R, name="wv", tag="wC")
                nc.gpsimd.dma_start(wv[:], w1v_d.ap()[ic])
                x3 = xp.rearrange("c (h w) -> c h w", h=34)
                xd4 = xd.rearrange("c (f h w) -> c f h w", f=4, h=17)
                first, last = (ic == 0), (ic == 15)
                for tap in range(9):
                    dy, dx = tap // 3, tap % 3
                    wq_t = wq[:, tap * 128:tap * 128 + 128]
                    for t in range(4):
                        win = x3[:, 8 * t + dy: 8 * t + dy + 8, dx: dx + 64]
                        nc.tensor.matmul(q1_ps[:, 512 * t: 512 * t + 512], wq_t, win,
                                         start=(first and tap == 0),
                                         stop=(last and tap == 8))
                for tap in range(9):
                    dy, dx = tap // 3, tap % 3
                    ph = 2 * (dy % 2) + (dx % 2)
                    win2 = xd4[:, ph:ph + 1, dy // 2: dy // 2 + 16, dx // 2: dx // 2 + 32]
                    nc.tensor.matmul(k1_ps[:], wk[:, tap * 128:tap * 128 + 128], win2,
                                     start=(first and tap == 0), stop=(last and tap == 8))
                    nc.tensor.matmul(v1_ps[:], wv[:, tap * 128:tap * 128 + 128], win2,
                                     start=(first and tap == 0), stop=(last and tap == 8))

            k1_sb = opool.tile([128, 15 * 33], F32R, name="k1_sb", tag="okv")
            k1o = k1_sb.rearrange("c (h w) -> c h w", h=15)
            k1g = k1_ps.rearrange("c (h w) -> c h w", h=16)
            _lrelu(nc, misc, k1g[:, 0:15, 0:31], bcol(1), bcol(11), k1o[:, :, 0:31], "k1e")
            nc.vector.tensor_copy(k1o[:, :, 31:33], k1o[:, :, 0:2])
            v1_sb = opool.tile([128, 15 * 33], F32R, name="v1_sb", tag="ovv")
            v1o = v1_sb.rearrange("c (h w) -> c h w", h=15)
            v1g = v1_ps.rearrange("c (h w) -> c h w", h=16)
            _lrelu(nc, misc, v1g[:, 0:15, 0:31], bcol(2), bcol(12), v1o[:, :, 0:31], "v1e")
            nc.vector.tensor_copy(v1o[:, :, 31:33], v1o[:, :, 0:2])

            q1_sb = opool.tile([128, 34 * 66], F32R, name="q1_sb", tag="obig")
            q1o = q1_sb.rearrange("c (h w) -> c h w", h=34)
            q1v = q1_ps.rearrange("c (h w) -> c h w", h=32)
            _lrelu(nc, misc, q1v[:, 0:16, :], bcol(0), bcol(10), q1o[:, 1:17, 1:65], "q1e0")
            _lrelu(nc, misc, q1v[:, 16:32, :], bcol(0), bcol(10), q1o[:, 17:33, 1:65], "q1e1")
            nc.vector.tensor_copy(q1o[:, 0:1, 1:65], q1o[:, 2:3, 1:65])
            nc.vector.tensor_copy(q1o[:, 33:34, 1:65], q1o[:, 31:32, 1:65])
            nc.vector.tensor_copy(q1o[:, :, 0:1], q1o[:, :, 64:65])
            nc.vector.tensor_copy(q1o[:, :, 65:66], q1o[:, :, 1:2])


            # k2/v2 partial convs + RSa, issued now so they overlap the q2 partials
            w2k = opool.tile([128, 2304], F32R, name="w2k", tag="wk2")
            nc.gpsimd.dma_start(w2k[:], w2k_d.ap())
            for cc in range(2):
                kp = ppool.tile([128, 112], F32, name="kp", tag="pk")
                for tap in range(9):
                    dy, dx = tap // 3, tap % 3
                    wink = k1o[:, dy: dy + 13: 2, dx: dx + 31: 2]
                    nc.tensor.matmul(kp[:], w2k[:, tap * 256 + 128 * cc: tap * 256 + 128 * cc + 128],
                                     wink, start=(tap == 0), stop=(tap == 8))
                kps = misc.tile([128, 112], F32, name="kps", tag="rss")
                nc.scalar.copy(kps[:], kp[:])
                dst = rsa_in[4 * cc:4 * cc + 4, 0:3584].rearrange("r (c p) -> r c p", c=32)
                nc.sync.dma_start(dst, kps[:])
            for vh in range(2):
                w2v = opool.tile([128, 4608], F32R, name="w2v", tag="wbig")
                nc.gpsimd.dma_start(w2v[:], w2v_d.ap()[vh])
                for cc4 in range(4):
                    cc = 4 * vh + cc4
                    vp = ppool.tile([128, 112], F32, name="vp", tag="pv")
                    for tap in range(9):
                        dy, dx = tap // 3, tap % 3
                        winv = v1o[:, dy: dy + 13: 2, dx: dx + 31: 2]
                        nc.tensor.matmul(vp[:], w2v[:, tap * 512 + 128 * cc4: tap * 512 + 128 * cc4 + 128],
                                         winv, start=(tap == 0), stop=(tap == 8))
                    vps = misc.tile([128, 112], F32, name="vps", tag="rss")
                    nc.scalar.copy(vps[:], vp[:])
                    nc.sync.dma_start(rsa_in[cc, 3584:17920].rearrange("(c p) -> c p", c=128), vps[:])
            nc.gpsimd.collective_compute("ReduceScatter", ALU.add, replica_groups=RG,
                                         ins=[rsa_in.opt()], outs=[rsa_out.opt()])

            # k2/v2 shard epilogues + AG2a (all overlap the q1 pass below)
            k2r = misc.tile([32, 112], F32, name="k2r", tag="rsl")
            nc.sync.dma_start(k2r[:], rsa_out[0:3584].rearrange("(c p) -> c p", c=32))
            v2r = misc.tile([128, 112], F32, name="v2r", tag="rsl2")
            nc.sync.dma_start(v2r[:], rsa_out[3584:17920].rearrange("(c p) -> c p", c=128))
            k2_sb = opool.tile([32, 112], F32R, name="k2_sb", tag="okv2")
            k2o = k2_sb.rearrange("c (h w) -> c h w", h=7)
            k2rg = k2r.rearrange("c (h w) -> c h w", h=7)
            _lrelu(nc, misc, k2rg[:, :, 0:15], bcol(4)[0:32], bcol(14)[0:32], k2o[:, :, 0:15], "k2e")
            nc.vector.tensor_copy(k2o[:, :, 15:16], k2o[:, :, 0:1])
            v2_sb = opool.tile([128, 112], F32R, name="v2_sb", tag="ovv2")
            v2o = v2_sb.rearrange("c (h w) -> c h w", h=7)
            v2rg = v2r.rearrange("c (h w) -> c h w", h=7)
            _lrelu(nc, misc, v2rg[:, :, 0:15], bcol(5), bcol(15), v2o[:, :, 0:15], "v2e")
            nc.vector.tensor_copy(v2o[:, :, 15:16], v2o[:, :, 0:1])
            nc.sync.dma_start(ag2a_in[0:3584].rearrange("(c p) -> c p", c=32), k2_sb[:])
            nc.sync.dma_start(ag2a_in[3584:17920].rearrange("(c p) -> c p", c=128), v2_sb[:])
            nc.gpsimd.collective_compute("AllGather", ALU.bypass, replica_groups=RG,
                                         ins=[ag2a_in.opt()], outs=[ag2a_out.opt()])

            # q2 partials (M=128!) from local q1
            w2q = opool.tile([128, 2304], F32R, name="w2q", tag="wq2")
            nc.gpsimd.dma_start(w2q[:], w2q_d.ap())
            for cc in range(2):
                qp = ppool.tile([128, 2048], F32, name="qp", tag="pbig")
                for tap in range(9):
                    dy, dx = tap // 3, tap % 3
                    wslc = w2q[:, tap * 256 + 128 * cc: tap * 256 + 128 * cc + 128]
                    for t in range(4):
                        win = q1o[:, 8 * t + dy: 8 * t + dy + 8, dx: dx + 64]
                        nc.tensor.matmul(qp[:, 512 * t:512 * t + 512], wslc, win,
                                         start=(tap == 0), stop=(tap == 8))
                qps = misc.tile([128, 2048], F32, name="qps", tag="rssb")
                nc.scalar.copy(qps[:], qp[:])
                dst = rsb_in[4 * cc:4 * cc + 4, :].rearrange("r (c p) -> r c p", c=32)
                nc.sync.dma_start(dst, qps[:])
            nc.gpsimd.collective_compute("ReduceScatter", ALU.add, replica_groups=RG,
                                         ins=[rsb_in.opt()], outs=[rsb_out.opt()])

            q2r = misc.tile([32, 2048], F32, name="q2r", tag="rssb")
            nc.sync.dma_start(q2r[:], rsb_out[:].rearrange("(c p) -> c p", c=32))
            q2_sb = opool.tile([32, 34 * 66], F32R, name="q2_sb", tag="obig")
            q2o = q2_sb.rearrange("c (h w) -> c h w", h=34)
            q2rv = q2r.rearrange("c (h w) -> c h w", h=32)
            _lrelu(nc, misc, q2rv[:, 0:16, :], bcol(3)[0:32], bcol(13)[0:32], q2o[:, 1:17, 1:65], "q2e0")
            _lrelu(nc, misc, q2rv[:, 16:32, :], bcol(3)[0:32], bcol(13)[0:32], q2o[:, 17:33, 1:65], "q2e1")
            nc.vector.tensor_copy(q2o[:, 0:1, 1:65], q2o[:, 2:3, 1:65])
            nc.vector.tensor_copy(q2o[:, 33:34, 1:65], q2o[:, 31:32, 1:65])
            nc.vector.tensor_copy(q2o[:, :, 0:1], q2o[:, :, 64:65])
            nc.vector.tensor_copy(q2o[:, :, 65:66], q2o[:, :, 1:2])
            nc.sync.dma_start(ag2b_in[:], q2_sb[:])
            nc.gpsimd.collective_compute("AllGather", ALU.bypass, replica_groups=RG,
                                         ins=[ag2b_in.opt()], outs=[ag2b_out.opt()])

            # projection weights: load+convert early so they overlap stage 3
            wpts = []
            for half in range(2):
                wpt = opool.tile([128, 4096], F32R, name="wpt", tag=f"wpt{half}")
                for qtr in range(2):
                    wstg = misc.tile([128, 2048], F32, name="wstg", tag="wstg")
                    nc.sync.dma_start(wstg.rearrange("b (a c) -> b a c", a=2),
                                      wp_d.ap()[4 * half + 2 * qtr: 4 * half + 2 * qtr + 2].rearrange("a b c -> b a c"))
                    nc.vector.tensor_copy(wpt[:, 2048 * qtr:2048 * qtr + 2048], wstg[:])
                wpts.append(wpt)

            # ============ STAGE 3: k3/v3 first (need ag2a), then q3 ==========
            k3_ps = ppool.tile([32, 70], F32, name="k3_ps", tag="pk")
            v3_ps = ppool.tile([128, 70], F32, name="v3_ps", tag="pv")
            k2rr = ag2a_out[:, 0:3584].rearrange("r (c p) -> r c p", c=32)
            v2rr = ag2a_out[:, 3584:17920].rearrange("r (c p) -> r c p", c=128)
            for jc in range(2):
                k2c = xpool.tile([128, 112], F32R, name="k2c", tag="k2c")
                nc.sync.dma_start(k2c[:], k2rr[4 * jc:4 * jc + 4])
                k2c3 = k2c.rearrange("c (h w) -> c h w", h=7)
                w3k = wpool.tile([128, 288], F32R, name="w3k", tag="wB")
                nc.gpsimd.dma_start(w3k[:], w3k_d.ap()[jc])
                for tap in range(9):
                    dy, dx = tap // 3, tap % 3
                    # k3: out grid (5, 14), cols 0..12 valid; N=70
                    wink = k2c3[:, dy: dy + 5, dx: dx + 14]
                    nc.tensor.matmul(k3_ps[:], w3k[:, tap * 32:tap * 32 + 32], wink,
                                     start=(jc == 0 and tap == 0), stop=(jc == 1 and tap == 8))
            for ic in range(8):
                v2c = xpool.tile([128, 112], F32R, name="v2c", tag="v2c")
                nc.sync.dma_start(v2c[:], v2rr[ic])
                v2c3 = v2c.rearrange("c (h w) -> c h w", h=7)
                w3v = wpool.tile([128, 1152], F32R, name="w3v", tag="wC")
                nc.gpsimd.dma_start(w3v[:], w3v_d.ap()[ic])
                for tap in range(9):
                    dy, dx = tap // 3, tap % 3
                    winv = v2c3[:, dy: dy + 5, dx: dx + 14]
                    nc.tensor.matmul(v3_ps[:], w3v[:, tap * 128:tap * 128 + 128], winv,
                                     start=(ic == 0 and tap == 0), stop=(ic == 7 and tap == 8))

            k3g = k3_ps.rearrange("c (h w) -> c h w", h=5)
            k3_sb = opool.tile([32, 65], F32, name="k3_sb", tag="okv")
            _lrelu(nc, misc, k3g[:, :, 0:13], bcol(7)[0:32], bcol(17)[0:32], k3_sb[:], "k3e")
            v3g = v3_ps.rearrange("c (h w) -> c h w", h=5)
            v3_sb = opool.tile([128, 65], F32, name="v3_sb", tag="v3sb")
            _lrelu(nc, misc, v3g[:, :, 0:13], bcol(8), bcol(18), v3_sb[:], "v3e")

            q3_ps = ppool.tile([32, 2048], F32, name="q3_ps", tag="pbig")
            for jc in range(2):
                q2p_t = xpool.tile([128, 34 * 66], F32R, name="q2p", tag="xbig")
                nc.sync.dma_start(q2p_t[:], ag2b_out[128 * jc:128 * jc + 128])
                q2p = q2p_t.rearrange("c (h w) -> c h w", h=34)
                w3q = wpool.tile([128, 288], F32R, name="w3q", tag="wA")
                nc.gpsimd.dma_start(w3q[:], w3q_d.ap()[jc])
                first, last = (jc == 0), (jc == 1)
                for tap in range(9):
                    dy, dx = tap // 3, tap % 3
                    for t in range(4):
                        win = q2p[:, 8 * t + dy: 8 * t + dy + 8, dx: dx + 64]
                        nc.tensor.matmul(q3_ps[:, 512 * t:512 * t + 512],
                                         w3q[:, tap * 32:tap * 32 + 32], win,
                                         start=(first and tap == 0), stop=(last and tap == 8))

            q3_sb = opool.tile([32, 2048], F32, name="q3_sb", tag="obig")
            _lrelu(nc, misc, q3_ps[:], bcol(6)[0:32], bcol(16)[0:32], q3_sb[:], "q3e")
            nc.sync.dma_start(ag3_in[:, 0:2048], q3_sb[:])
            nc.sync.dma_start(ag3_in[:, 2048:2113], k3_sb[:])
            # v3 packed as channel p -> (row p//4, col-block p%4)
            nc.sync.dma_start(ag3_in[:, 2113:2373].rearrange("c (a p) -> c a p", a=4), v3_sb[:])
            nc.gpsimd.collective_compute("AllGather", ALU.bypass, replica_groups=RG,
                                         ins=[ag3_in.opt()], outs=[ag3_out.opt()])

            # ============ STAGE 4: attention + position-sharded projection ===
            sc_ps = ppool.tile([65, 2048], F32, name="sc_ps", tag="pbig")
            for jc in range(2):
                q3f = opool.tile([128, 2048], F32, name="q3f", tag="wq2")
                nc.sync.dma_start(q3f[:], ag3_out[128 * jc:128 * jc + 128, 0:2048])
                k3f = misc.tile([128, 65], F32, name="k3f", tag="k3f", bufs=2)
                nc.sync.dma_start(k3f[:], ag3_out[128 * jc:128 * jc + 128, 2048:2113])
                for t in range(4):
                    nc.tensor.matmul(sc_ps[:, 512 * t:512 * t + 512], k3f[:],
                                     q3f[:, 512 * t:512 * t + 512],
                                     start=(jc == 0), stop=(jc == 1))

            negmax = misc.tile([65, 1], F32)
            nc.vector.reduce_max(negmax[:], sc_ps[:], axis=AX.X, negate=True)
            esum = misc.tile([65, 1], F32)
            bexp = misc.tile([65, 2048], F32)
            nc.scalar.activation(bexp[:], sc_ps[:], AF.Exp, bias=negmax[:, 0:1],
                                 accum_out=esum[:, 0:1])
            rsum = misc.tile([65, 1], F32)
            nc.vector.reciprocal(rsum[:], esum[:])
            # ship UNnormalized exp; the 1/sum(m) factor is folded into v3^T
            # below (commutes through the m-contraction of the o matmul)
            nc.sync.dma_start(beta_dram[:], bexp[:])


            # indirect gather of MY 256 beta columns: row (m, blk) of (520, 256)
            bidx = misc.tile([65, 1], I32)
            nc.sync.dma_start(bidx[:], bidx_d.ap())
            betaB = misc.tile([65, 256], F32)
            nc.gpsimd.indirect_dma_start(
                out=betaB[:], out_offset=None,
                in_=beta_dram.rearrange("m (b p) -> (m b) p", b=8),
                in_offset=bass.IndirectOffsetOnAxis(ap=bidx[:, 0:1], axis=0))

            # v3^T chunks (65, 128) for all 1024 v-channels
            ident = misc.tile([128, 128], F32)
            nc.sync.dma_start(ident[:], ident_d.ap())
            v3r = ag3_out[:, 2113:2373].rearrange("r (a p) -> r a p", a=4)
            v3ta = misc.tile([65, 1024], F32, name="v3ta")
            for i in range(8):
                v3f = misc.tile([128, 65], F32, name="v3f", tag="v3f", bufs=2)
                nc.sync.dma_start(v3f[:], v3r[32 * i:32 * i + 32])
                tps = ppool.tile([65, 128], F32, name="tps", tag="pk")
                nc.tensor.transpose(tps[:], v3f[:, 0:65], ident[:])
                nc.scalar.copy(v3ta[:, 128 * i:128 * i + 128], tps[:])
            nc.vector.tensor_scalar_mul(v3ta[:], v3ta[:], rsum[:, 0:1])

            # o chunks (128 v-ch, 256 pos) then projection (all 1024 out-ch)
            oia = misc.tile([128, 2048], F32R, name="oia", tag="rssb")
            for i in range(8):
                ops = ppool.tile([128, 256], F32, name="ops", tag="pv")
                nc.tensor.matmul(ops[:], v3ta[:, 128 * i:128 * i + 128], betaB[:],
                                 start=True, stop=True)
                nc.scalar.copy(oia[:, 256 * i:256 * i + 256], ops[:])
            acca = misc.tile([128, 2048], F32, name="acca", tag="bexp")
            for half in range(2):
                wpt = wpts[half]
                for cc in range(8):
                    out_ps = ppool.tile([128, 256], F32, name="out_ps", tag="pk")
                    for c4 in range(4):
                        cik = 4 * half + c4
                        nc.tensor.matmul(out_ps[:],
                                         wpt[:, 1024 * c4 + 128 * cc: 1024 * c4 + 128 * cc + 128],
                                         oia[:, 256 * cik:256 * cik + 256],
                                         start=(c4 == 0), stop=(c4 == 3))
                    if half == 0:
                        nc.vector.tensor_scalar_add(acca[:, 256 * cc:256 * cc + 256],
                                                    out_ps[:], bcol(20 + cc))
                    else:
                        out_sb = misc.tile([128, 256], F32, name="out_sb", tag="osb", bufs=2)
                        nc.vector.tensor_tensor(out_sb[:], acca[:, 256 * cc:256 * cc + 256],
                                                out_ps[:], op=ALU.add)
                        nc.sync.dma_start(out_d.ap()[128 * cc:128 * cc + 128], out_sb[:])

    nc.compile()
    nc.m = get_hw_module(nc.m)
    return nc


def _prep_inputs(x, qw1, qb1, qw2, qb2, qw3, qb3, kw1, kb1, kw2, kb2, kw3, kb3,
                 vw1, vb1, vw2, vb2, vw3, vb3, pw, pb):
    f = np.float32
    x = np.ascontiguousarray(np.asarray(x).reshape(2048, 32, 64), dtype=f)
    xp = np.concatenate([x[:, 1:2], x, x[:, 30:31]], axis=1)
    xp = np.concatenate([xp[:, :, -1:], xp, xp[:, :, :1]], axis=2)
    xpad = np.ascontiguousarray(xp.reshape(16, 128, 34 * 66))
    xdec = np.zeros((16, 128, 4, 17, 33), f)
    xr = x.reshape(16, 128, 32, 64)
    for py in range(2):
        for px in range(2):
            xdec[:, :, 2 * py + px, 0:16, 0:32] = xr[:, :, py::2, px::2]
    xdec = np.ascontiguousarray(xdec.reshape(16, 128, 4 * 17 * 33))

    def conv_w(wt, co_lo, co_n, nchunk):
        ws = np.asarray(wt)[co_lo:co_lo + co_n]           # (co_n, Ci, 3, 3)
        ci = ws.shape[1]
        a = ws.reshape(co_n, nchunk, ci // nchunk, 9)     # (co, ck, ci, tap)
        a = a.transpose(1, 2, 3, 0)                       # (ck, ci, tap, co)
        return np.ascontiguousarray(a.reshape(nchunk, ci // nchunk, 9 * co_n), dtype=f)

    def conv_w_ci(wt, ci_lo):
        # full out-channels, my 128 input channels -> (128ci, 9*co)
        ws = np.asarray(wt)[:, ci_lo:ci_lo + 128]         # (co, 128, 3, 3)
        co = ws.shape[0]
        a = ws.reshape(co, 128, 9).transpose(1, 2, 0)     # (ci, tap, co)
        return np.ascontiguousarray(a.reshape(128, 9 * co), dtype=f)

    in_maps = []
    for c in range(NCORES):
        m = {"xpad": xpad, "xdec": xdec}
        m["w1q"] = conv_w(qw1, 128 * c, 128, 16)
        m["w1k"] = conv_w(kw1, 128 * c, 128, 16)
        m["w1v"] = conv_w(vw1, 128 * c, 128, 16)
        m["w2q"] = conv_w_ci(qw2, 128 * c)
        m["w2k"] = conv_w_ci(kw2, 128 * c)
        wv2 = np.asarray(vw2)[:, 128 * c:128 * c + 128]        # (1024co, 128ci, 3, 3)
        wv2 = wv2.reshape(2, 512, 128, 9).transpose(0, 2, 3, 1)  # (half, ci, tap, co512)
        m["w2v"] = np.ascontiguousarray(wv2.reshape(2, 128, 4608), dtype=f)
        m["w3q"] = conv_w(qw3, 32 * c, 32, 2)
        m["w3k"] = conv_w(kw3, 32 * c, 32, 2)
        m["w3v"] = conv_w(vw3, 128 * c, 128, 8)
        m["wp"] = np.ascontiguousarray(
            np.asarray(pw)[:, :, 0, 0].T.reshape(8, 128, 1024), dtype=f)
        bias = np.zeros((128, 28), f)
        bias[:, 0] = qb1[128 * c:128 * c + 128]
        bias[:, 1] = kb1[128 * c:128 * c + 128]
        bias[:, 2] = vb1[128 * c:128 * c + 128]
        bias[0:32, 3] = qb2[32 * c:32 * c + 32]
        bias[0:32, 4] = kb2[32 * c:32 * c + 32]
        bias[:, 5] = vb2[128 * c:128 * c + 128]
        bias[0:32, 6] = qb3[32 * c:32 * c + 32]
        bias[0:32, 7] = kb3[32 * c:32 * c + 32]
        bias[:, 8] = vb3[128 * c:128 * c + 128]
        bias[:, 10:19] = 0.3 * bias[:, 0:9]
        for j in range(8):
            bias[:, 20 + j] = pb[128 * j:128 * j + 128]
        m["bias"] = bias
        m["bidx"] = np.arange(65, dtype=np.int32).reshape(65, 1) * 8 + c
        in_maps.append(m)
    return in_maps


LAST_RESULT = None


def kernel(**inputs):
    global LAST_RESULT
    if "nc" not in _CACHE:
        _CACHE["nc"] = build_program()
    nc = _CACHE["nc"]
    in_maps = _prep_inputs(**{k: np.asarray(v) for k, v in inputs.items()})
    res = bass_utils.run_bass_kernel_spmd(nc, in_maps, core_ids=list(range(NCORES)))
    LAST_RESULT = res
    out = np.empty((1024, 32, 64), np.float32)
    for c in range(NCORES):
        out[:, 4 * c:4 * c + 4, :] = res.results[c]["out_shard"].reshape(1024, 4, 64)
    return np.ascontiguousarray(out.reshape(1, 1024, 32, 64))
